# revision 3
# baseline (speedup 1.0000x reference)
"""Trainium2 Bass kernel for a 3-layer binarized CNN (tunnel-optimized).

Network (reference):
    x  : [32, 3, 512, 512] fp32
    l1 : clip(conv(x, sign(w1)))            -> [32,16,510,510]
    l2 : clip(conv(sign(l1), sign(w2)))     -> [32,23,508,508]
    l3 : clip(conv(sign(l2), sign(w3)))     -> [32,2,506,506]
    out: l3.reshape(32, -1)

The axon tunnel moves ~45 MB/s, so end-to-end wall time is dominated by
bytes shipped, not device cycles.  Strategy:

  * Layer 1 only matters through the SIGN of its fp32 output.  It is
    computed on the host (one 16x27 sgemm per image) and shipped as
    bit-packed signs: 17 MB instead of the 100 MB fp32 input.
  * The device unpacks bits to +-1 fp8 activations (DVE shift/and ops),
    then runs layers 2+3 exactly as integer-exact fp8 DoubleRow
    Toeplitz matmuls (4 images per core x 8 cores, data parallel).
  * The output (values in {-1,0,1}) is packed 4 columns/byte as
    balanced-quaternary digits: p = s0+4*s1+16*s2+64*s3 in int8, cutting
    the returned tensor (and its donated zero-buffer upload) 4x.
    The host decodes with a 256x4 LUT.
"""

import numpy as np
from numpy.lib.stride_tricks import as_strided

import concourse.bacc as bacc
import concourse.mybir as mybir
import concourse.tile as tile
from concourse.bass_utils import run_bass_kernel_spmd

F32 = mybir.dt.float32
F16 = mybir.dt.float16
F8 = mybir.dt.float8e4
U8 = mybir.dt.uint8
I8 = mybir.dt.int8
ALU = mybir.AluOpType
DR = mybir.MatmulPerfMode.DoubleRow
SIGN = mybir.ActivationFunctionType.Sign

N_CORES = 8
AL2, AO2 = 7, 5      # L2: rows window / rows out per block
AL3, AO3 = 32, 30    # L3
C2, C3 = 16, 23
O2, O3 = 23, 2


def _toeplitz_weights(w2, w3):
    """Build the stationary Toeplitz matrices (host side)."""
    s2 = np.sign(w2).astype(np.float32)  # [23,16,3,3]
    s3 = np.sign(w3).astype(np.float32)  # [2,23,3,3]

    # T2[(al*8 + cp), dx, codd, (aol*23 + o)] fp8 DoubleRow pairs, M pad 128
    t2 = np.zeros((56, 3, 2, 128), np.float32)
    for al in range(AL2):
        for aol in range(AO2):
            dy = al - aol
            if 0 <= dy <= 2:
                for c in range(C2):
                    for o in range(O2):
                        t2[al * 8 + c // 2, :, c % 2, aol * 23 + o] = s2[o, c, dy, :]
    # T3[(al*4 + cp), cc, dx, codd, (o*30 + aol)] fp8 DoubleRow, M pad 64.
    # M index is o-major so the output lands channel-major in DRAM.
    t3 = np.zeros((128, 3, 3, 2, 64), np.float32)
    for al in range(AL3):
        for aol in range(AO3):
            dy = al - aol
            if 0 <= dy <= 2:
                for cc in range(3):
                    for cl in range(8):
                        c = cc * 8 + cl
                        if c < C3:
                            for o in range(O3):
                                t3[al * 4 + cl // 2, cc, :, cl % 2, o * 30 + aol] = (
                                    s3[o, c, dy, :]
                                )
    import ml_dtypes

    return (
        t2.reshape(56, 3 * 256).astype(ml_dtypes.float8_e4m3),
        t3.reshape(128, 9 * 128).astype(ml_dtypes.float8_e4m3),
    )


def _build_program(n_img, A, B):
    """Emit the per-core SPMD Bass program (unpack + layers 2,3)."""
    n1, n2, n3 = B - 2, B - 4, B - 6          # 510, 508, 506
    r1 = A - 2                                 # 510 rows of l1 signs
    nblk2 = -(-(A - 4) // AO2)                 # 102
    nblk3 = -(-(A - 6) // AO3)                 # 17
    a1 = AL2 + AO2 * (nblk2 - 1)               # 512 s1 rows (incl zero tail)
    s2a = max(AO2 * nblk2, AO3 * (nblk3 - 1) + AL3)  # 512
    npk = (n3 + 2) // 4                        # 127 packed output bytes

    nc = bacc.Bacc("TRN2", target_bir_lowering=False, debug=False)

    s1p = nc.dram_tensor("s1p", [n_img, r1, 16, 64], U8, kind="ExternalInput")
    t2w = nc.dram_tensor("t2w", [56, 3 * 256], F8, kind="ExternalInput")
    t3w = nc.dram_tensor("t3w", [128, 9 * 128], F8, kind="ExternalInput")
    outp = nc.dram_tensor(
        "outp", [n_img, 2, AO3 * nblk3, npk], I8, kind="ExternalOutput"
    )
    s1d = [
        nc.dram_tensor(f"s1_{i}", [a1, 16, n1], F8, kind="Internal")
        for i in range(n_img)
    ]
    s2d = [
        nc.dram_tensor(f"s2_{i}", [s2a, 24, n2], F8, kind="Internal")
        for i in range(n_img)
    ]

    with tile.TileContext(nc) as tc:
        with (
            tc.tile_pool(name="const", bufs=1) as cpool,
            tc.tile_pool(name="unp", bufs=3) as pu,
            tc.tile_pool(name="l2", bufs=4) as p2,
            tc.tile_pool(name="l3", bufs=8) as p3,
            tc.tile_pool(name="ps2", bufs=3, space="PSUM") as ps2p,
            tc.tile_pool(name="ps3", bufs=3, space="PSUM") as ps3p,
        ):
            t2sb = cpool.tile([56, 3 * 256], F8)
            t3sb = cpool.tile([128, 9 * 128], F8)
            ztile = cpool.tile([128, B], F8)
            nc.sync.dma_start(t2sb[:], t2w.ap()[:])
            nc.sync.dma_start(t3sb[:], t3w.ap()[:])
            nc.vector.memset(ztile[:], 0.0)

            for img in range(n_img):
                s1, s2 = s1d[img].ap(), s2d[img].ap()
                # ---- zero pads of s2: channel-23 plane + tail rows ----
                for r in range(0, s2a, 128):
                    cnt = min(128, s2a - r)
                    nc.sync.dma_start(s2[r : r + cnt, 23, :], ztile[:cnt, :n2])
                for a in range(AO2 * nblk2, s2a):
                    nc.sync.dma_start(s2[a, :, :], ztile[:24, :n2])
                # ---- zero tail rows of s1 (rows r1..a1) ----
                if a1 > r1:
                    nc.sync.dma_start(
                        s1[r1:a1].rearrange("r c n -> (r c) n"),
                        ztile[: 16 * (a1 - r1), :n1],
                    )

                # ---------- unpack host L1 sign bits -> s1 (+-1 fp8) ----------
                for r0 in range(0, r1, 16):
                    cnt = min(16, r1 - r0)
                    np_ = cnt * 8
                    pk = pu.tile([128, 128], U8, tag="pk")
                    nc.sync.dma_start(
                        pk[:np_, :],
                        s1p.ap()[img, r0 : r0 + cnt].rearrange(
                            "r (c t) b -> (r c) (t b)", t=2
                        ),
                    )
                    upk = pu.tile([128, 2 * 512], F8, tag="upk")
                    upkv = upk[:].rearrange("p (t n f) -> p t n f", t=2, f=8)
                    pkv = pk[:].rearrange("p (t n) -> p t n", t=2)
                    for k in range(8):
                        tb = pu.tile([128, 128], U8, tag="tb")
                        nc.vector.tensor_scalar(
                            tb[:np_, :], pk[:np_, :], int(7 - k), int(1),
                            op0=ALU.logical_shift_right, op1=ALU.bitwise_and,
                        )
                        tbv = tb[:].rearrange("p (t n) -> p t n", t=2)
                        nc.vector.tensor_scalar(
                            upkv[:np_, :, :, k], tbv[:np_, :, :], 2.0, -1.0,
                            op0=ALU.mult, op1=ALU.add,
                        )
                    nc.sync.dma_start(
                        s1[r0 : r0 + cnt].rearrange("r (c t) n -> (r c) t n", t=2),
                        upk[:np_].rearrange("p (t n) -> p t n", t=2)[:, :, 0:n1],
                    )

                # ---------------- layer 2 (fp8 DoubleRow) ----------------
                for b in range(nblk2):
                    rhs8 = p2.tile([56, 2 * 512], F8, tag="rhs8")
                    r3 = rhs8[:].rearrange("k (t h) -> k t h", t=2)
                    nc.sync.dma_start(r3[:, :, 0:n1], s1[5 * b : 5 * b + 7, :, :])
                    ps = ps2p.tile([115, n2], F32, tag="ps2")
                    for dx in range(3):
                        nc.tensor.matmul(
                            ps[:],
                            t2sb[:, 256 * dx : 256 * dx + 256].rearrange(
                                "k (t m) -> k t m", t=2
                            )[:, :, 0:115],
                            r3[:, :, dx : dx + n2],
                            start=(dx == 0),
                            stop=(dx == 2),
                            perf_mode=DR,
                        )
                    sg2 = p2.tile([115, n2], F8, tag="sg2")
                    nc.scalar.activation(sg2[:], ps[:], SIGN)
                    nc.sync.dma_start(s2[5 * b : 5 * b + 5, 0:23, :], sg2[:])

                # -------- layer 3 (fp8 DoubleRow) + 2-bit output pack --------
                for bb in range(nblk3):
                    rts = []
                    for cc in range(3):
                        rt = p3.tile([128, 2 * 512], F8, tag="rhs3")
                        nc.sync.dma_start(
                            rt[:].rearrange("k (t h) -> k t h", t=2)[:, :, 0:n2],
                            s2[30 * bb : 30 * bb + 32, 8 * cc : 8 * cc + 8, :],
                        )
                        rts.append(rt)
                    ps = ps3p.tile([60, n3], F32, tag="ps3")
                    for cc in range(3):
                        for dx in range(3):
                            nc.tensor.matmul(
                                ps[:],
                                t3sb[
                                    :, 128 * (cc * 3 + dx) : 128 * (cc * 3 + dx) + 128
                                ].rearrange("k (t m) -> k t m", t=2)[:, :, 0:60],
                                rts[cc][:].rearrange("k (t h) -> k t h", t=2)[
                                    :, :, dx : dx + n3
                                ],
                                start=(cc == 0 and dx == 0),
                                stop=(cc == 2 and dx == 2),
                                perf_mode=DR,
                            )
                    sg = p3.tile([60, 4 * npk], F16, tag="sg")
                    nc.vector.memset(sg[:, n3 : 4 * npk], 0.0)
                    nc.scalar.activation(sg[:, 0:n3], ps[:], SIGN)
                    sgv = sg[:].rearrange("p (n f) -> p n f", f=4)
                    pa = p3.tile([60, npk], F16, tag="pa")
                    nc.vector.scalar_tensor_tensor(
                        pa[:], sgv[:, :, 1], 4.0, sgv[:, :, 0],
                        op0=ALU.mult, op1=ALU.add,
                    )
                    pb = p3.tile([60, npk], F16, tag="pb")
                    nc.vector.scalar_tensor_tensor(
                        pb[:], sgv[:, :, 2], 16.0, pa[:],
                        op0=ALU.mult, op1=ALU.add,
                    )
                    po = p3.tile([60, npk], I8, tag="po")
                    nc.vector.scalar_tensor_tensor(
                        po[:], sgv[:, :, 3], 64.0, pb[:],
                        op0=ALU.mult, op1=ALU.add,
                    )
                    for o in range(2):
                        nc.sync.dma_start(
                            outp.ap()[img, o, 30 * bb : 30 * bb + 30, :],
                            po[30 * o : 30 * o + 30, :],
                        )

    nc.compile()
    return nc


_CACHE = {}


def _get_program(n_img, A, B):
    key = (n_img, A, B)
    if key not in _CACHE:
        _CACHE[key] = _build_program(n_img, A, B)
    return _CACHE[key]


def _host_l1_pack(x, w1):
    """conv(x, sign(w1)) > 0, bit-packed as [n, 510, 16, 64] uint8."""
    n = x.shape[0]
    W = np.sign(w1).astype(np.float32).reshape(16, 27)
    s = x.strides
    packed = np.empty((n, 510, 16, 64), np.uint8)
    for i in range(n):
        xv = as_strided(
            x[i], (3, 3, 3, 510, 510), (s[1], s[2], s[3], s[2], s[3])
        )
        col = np.ascontiguousarray(xv.reshape(27, -1))
        y = W @ col
        b = (y > 0).reshape(16, 510, 510)
        packed[i] = np.packbits(b.transpose(1, 0, 2), axis=-1)
    return packed


# decode LUT: int8 byte p = s0 + 4*s1 + 16*s2 + 64*s3 (balanced quaternary)
_LUT = np.zeros((256, 4), np.float32)
for _s3 in (-1, 0, 1):
    for _s2 in (-1, 0, 1):
        for _s1 in (-1, 0, 1):
            for _s0 in (-1, 0, 1):
                _p = _s0 + 4 * _s1 + 16 * _s2 + 64 * _s3
                _LUT[_p + 128] = (_s0, _s1, _s2, _s3)

last_results = None


def kernel(inputs, w1, w2, w3):
    global last_results
    x = np.asarray(inputs, np.float32)
    n, _, A, B = x.shape
    per = n // N_CORES
    nc = _get_program(per, A, B)
    t2, t3 = _toeplitz_weights(
        np.asarray(w2, np.float32), np.asarray(w3, np.float32)
    )
    packed = _host_l1_pack(x, np.asarray(w1, np.float32))
    maps = [
        {"s1p": packed[per * i : per * (i + 1)], "t2w": t2, "t3w": t3}
        for i in range(N_CORES)
    ]
    res = run_bass_kernel_spmd(nc, maps, core_ids=list(range(N_CORES)))
    last_results = res
    a3, b3 = A - 6, B - 6
    out = np.empty((n, 2, a3, b3), np.float32)
    for i, r in enumerate(res.results):
        pk = r["outp"]  # [per, 2, 510, 127] int8
        idx = pk.astype(np.int16) + 128
        vals = _LUT[idx]  # [per, 2, 510, 127, 4]
        vals = vals.reshape(per, 2, pk.shape[2], -1)
        out[per * i : per * (i + 1)] = vals[:, :, :a3, :b3]
    return out.reshape(n, -1)


# revision 4
# speedup vs baseline: 1.3177x; 1.3177x over previous
"""Trainium2 Bass kernel for a 3-layer binarized CNN (tunnel-optimized).

Network (reference):
    x  : [32, 3, 512, 512] fp32
    l1 : clip(conv(x, sign(w1)))            -> [32,16,510,510]
    l2 : clip(conv(sign(l1), sign(w2)))     -> [32,23,508,508]
    l3 : clip(conv(sign(l2), sign(w3)))     -> [32,2,506,506]
    out: l3.reshape(32, -1)

The axon tunnel moves ~45 MB/s, so end-to-end wall time is dominated by
bytes shipped, not device cycles.  Strategy:

  * Layer 1 only matters through the SIGN of its fp32 output.  It is
    computed on the host (one 16x27 sgemm per image) and shipped as
    bit-packed signs: 17 MB instead of the 100 MB fp32 input.
  * The device unpacks bits to +-1 fp8 activations (DVE shift/and ops),
    then runs layers 2+3 exactly as integer-exact fp8 DoubleRow
    Toeplitz matmuls (4 images per core x 8 cores, data parallel).
  * The output (values in {-1,0,1}) is packed 4 columns/byte as
    balanced-quaternary digits: p = s0+4*s1+16*s2+64*s3 in int8, cutting
    the returned tensor (and its donated zero-buffer upload) 4x.
    The host decodes with a 256x4 LUT.
"""

import numpy as np
from numpy.lib.stride_tricks import as_strided

import concourse.bacc as bacc
import concourse.mybir as mybir
import concourse.tile as tile
from concourse.bass_utils import run_bass_kernel_spmd

F32 = mybir.dt.float32
F16 = mybir.dt.float16
F8 = mybir.dt.float8e4
U8 = mybir.dt.uint8
I8 = mybir.dt.int8
ALU = mybir.AluOpType
DR = mybir.MatmulPerfMode.DoubleRow
SIGN = mybir.ActivationFunctionType.Sign

N_CORES = 8
AL2, AO2 = 7, 5      # L2: rows window / rows out per block
AL3, AO3 = 32, 30    # L3
C2, C3 = 16, 23
O2, O3 = 23, 2


def _toeplitz_weights(w2, w3):
    """Build the stationary Toeplitz matrices (host side)."""
    s2 = np.sign(w2).astype(np.float32)  # [23,16,3,3]
    s3 = np.sign(w3).astype(np.float32)  # [2,23,3,3]

    # T2[(al*8 + cp), dx, codd, (aol*23 + o)] fp8 DoubleRow pairs, M pad 128
    t2 = np.zeros((56, 3, 2, 128), np.float32)
    for al in range(AL2):
        for aol in range(AO2):
            dy = al - aol
            if 0 <= dy <= 2:
                for c in range(C2):
                    for o in range(O2):
                        t2[al * 8 + c // 2, :, c % 2, aol * 23 + o] = s2[o, c, dy, :]
    # T3[(al*4 + cp), cc, dx, codd, (o*30 + aol)] fp8 DoubleRow, M pad 64.
    # M index is o-major so the output lands channel-major in DRAM.
    t3 = np.zeros((128, 3, 3, 2, 64), np.float32)
    for al in range(AL3):
        for aol in range(AO3):
            dy = al - aol
            if 0 <= dy <= 2:
                for cc in range(3):
                    for cl in range(8):
                        c = cc * 8 + cl
                        if c < C3:
                            for o in range(O3):
                                t3[al * 4 + cl // 2, cc, :, cl % 2, o * 30 + aol] = (
                                    s3[o, c, dy, :]
                                )
    import ml_dtypes

    return (
        t2.reshape(56, 3 * 256).astype(ml_dtypes.float8_e4m3),
        t3.reshape(128, 9 * 128).astype(ml_dtypes.float8_e4m3),
    )


def _build_program(n_img, A, B):
    """Emit the per-core SPMD Bass program (unpack + layers 2,3)."""
    n1, n2, n3 = B - 2, B - 4, B - 6          # 510, 508, 506
    r1 = A - 2                                 # 510 rows of l1 signs
    nblk2 = -(-(A - 4) // AO2)                 # 102
    nblk3 = -(-(A - 6) // AO3)                 # 17
    a1 = AL2 + AO2 * (nblk2 - 1)               # 512 s1 rows (incl zero tail)
    s2a = max(AO2 * nblk2, AO3 * (nblk3 - 1) + AL3)  # 512
    npk = (n3 + 2) // 4                        # 127 packed output bytes

    nc = bacc.Bacc("TRN2", target_bir_lowering=False, debug=False)

    s1p = nc.dram_tensor("s1p", [n_img, r1, 16, 64], U8, kind="ExternalInput")
    t2w = nc.dram_tensor("t2w", [56, 3 * 256], F8, kind="ExternalInput")
    t3w = nc.dram_tensor("t3w", [128, 9 * 128], F8, kind="ExternalInput")
    outp = nc.dram_tensor(
        "outp", [n_img, 2, AO3 * nblk3, npk], I8, kind="ExternalOutput"
    )
    s1d = [
        nc.dram_tensor(f"s1_{i}", [a1, 16, n1], F8, kind="Internal")
        for i in range(n_img)
    ]
    s2d = [
        nc.dram_tensor(f"s2_{i}", [s2a, 24, n2], F8, kind="Internal")
        for i in range(n_img)
    ]

    with tile.TileContext(nc) as tc:
        with (
            tc.tile_pool(name="const", bufs=1) as cpool,
            tc.tile_pool(name="unp", bufs=3) as pu,
            tc.tile_pool(name="l2", bufs=4) as p2,
            tc.tile_pool(name="l3", bufs=8) as p3,
            tc.tile_pool(name="ps2", bufs=3, space="PSUM") as ps2p,
            tc.tile_pool(name="ps3", bufs=3, space="PSUM") as ps3p,
        ):
            t2sb = cpool.tile([56, 3 * 256], F8)
            t3sb = cpool.tile([128, 9 * 128], F8)
            ztile = cpool.tile([128, B], F8)
            nc.sync.dma_start(t2sb[:], t2w.ap()[:])
            nc.sync.dma_start(t3sb[:], t3w.ap()[:])
            nc.vector.memset(ztile[:], 0.0)

            for img in range(n_img):
                s1, s2 = s1d[img].ap(), s2d[img].ap()
                # ---- zero pads of s2: channel-23 plane + tail rows ----
                for r in range(0, s2a, 128):
                    cnt = min(128, s2a - r)
                    nc.sync.dma_start(s2[r : r + cnt, 23, :], ztile[:cnt, :n2])
                for a in range(AO2 * nblk2, s2a):
                    nc.sync.dma_start(s2[a, :, :], ztile[:24, :n2])
                # ---- zero tail rows of s1 (rows r1..a1) ----
                if a1 > r1:
                    nc.sync.dma_start(
                        s1[r1:a1].rearrange("r c n -> (r c) n"),
                        ztile[: 16 * (a1 - r1), :n1],
                    )

                # ---------- unpack host L1 sign bits -> s1 (+-1 fp8) ----------
                for r0 in range(0, r1, 16):
                    cnt = min(16, r1 - r0)
                    np_ = cnt * 8
                    pk = pu.tile([128, 128], U8, tag="pk")
                    nc.sync.dma_start(
                        pk[:np_, :],
                        s1p.ap()[img, r0 : r0 + cnt].rearrange(
                            "r (c t) b -> (r c) (t b)", t=2
                        ),
                    )
                    upk = pu.tile([128, 2 * 512], F8, tag="upk")
                    upkv = upk[:].rearrange("p (t n f) -> p t n f", t=2, f=8)
                    pkv = pk[:].rearrange("p (t n) -> p t n", t=2)
                    for k in range(8):
                        tb = pu.tile([128, 128], U8, tag="tb")
                        nc.vector.tensor_scalar(
                            tb[:np_, :], pk[:np_, :], int(7 - k), int(1),
                            op0=ALU.logical_shift_right, op1=ALU.bitwise_and,
                        )
                        tbv = tb[:].rearrange("p (t n) -> p t n", t=2)
                        nc.vector.tensor_scalar(
                            upkv[:np_, :, :, k], tbv[:np_, :, :], 2.0, -1.0,
                            op0=ALU.mult, op1=ALU.add,
                        )
                    nc.sync.dma_start(
                        s1[r0 : r0 + cnt].rearrange("r (c t) n -> (r c) t n", t=2),
                        upk[:np_].rearrange("p (t n) -> p t n", t=2)[:, :, 0:n1],
                    )

                # ---------------- layer 2 (fp8 DoubleRow) ----------------
                for b in range(nblk2):
                    rhs8 = p2.tile([56, 2 * 512], F8, tag="rhs8")
                    r3 = rhs8[:].rearrange("k (t h) -> k t h", t=2)
                    nc.sync.dma_start(r3[:, :, 0:n1], s1[5 * b : 5 * b + 7, :, :])
                    ps = ps2p.tile([115, n2], F32, tag="ps2")
                    for dx in range(3):
                        nc.tensor.matmul(
                            ps[:],
                            t2sb[:, 256 * dx : 256 * dx + 256].rearrange(
                                "k (t m) -> k t m", t=2
                            )[:, :, 0:115],
                            r3[:, :, dx : dx + n2],
                            start=(dx == 0),
                            stop=(dx == 2),
                            perf_mode=DR,
                        )
                    sg2 = p2.tile([115, n2], F8, tag="sg2")
                    nc.scalar.activation(sg2[:], ps[:], SIGN)
                    nc.sync.dma_start(s2[5 * b : 5 * b + 5, 0:23, :], sg2[:])

                # -------- layer 3 (fp8 DoubleRow) + 2-bit output pack --------
                for bb in range(nblk3):
                    rts = []
                    for cc in range(3):
                        rt = p3.tile([128, 2 * 512], F8, tag="rhs3")
                        nc.sync.dma_start(
                            rt[:].rearrange("k (t h) -> k t h", t=2)[:, :, 0:n2],
                            s2[30 * bb : 30 * bb + 32, 8 * cc : 8 * cc + 8, :],
                        )
                        rts.append(rt)
                    ps = ps3p.tile([60, n3], F32, tag="ps3")
                    for cc in range(3):
                        for dx in range(3):
                            nc.tensor.matmul(
                                ps[:],
                                t3sb[
                                    :, 128 * (cc * 3 + dx) : 128 * (cc * 3 + dx) + 128
                                ].rearrange("k (t m) -> k t m", t=2)[:, :, 0:60],
                                rts[cc][:].rearrange("k (t h) -> k t h", t=2)[
                                    :, :, dx : dx + n3
                                ],
                                start=(cc == 0 and dx == 0),
                                stop=(cc == 2 and dx == 2),
                                perf_mode=DR,
                            )
                    sg = p3.tile([60, 4 * npk], F16, tag="sg")
                    nc.vector.memset(sg[:, n3 : 4 * npk], 0.0)
                    nc.scalar.activation(sg[:, 0:n3], ps[:], SIGN)
                    sgv = sg[:].rearrange("p (n f) -> p n f", f=4)
                    pa = p3.tile([60, npk], F16, tag="pa")
                    nc.vector.scalar_tensor_tensor(
                        pa[:], sgv[:, :, 1], 4.0, sgv[:, :, 0],
                        op0=ALU.mult, op1=ALU.add,
                    )
                    pb = p3.tile([60, npk], F16, tag="pb")
                    nc.vector.scalar_tensor_tensor(
                        pb[:], sgv[:, :, 2], 16.0, pa[:],
                        op0=ALU.mult, op1=ALU.add,
                    )
                    po = p3.tile([60, npk], I8, tag="po")
                    nc.vector.scalar_tensor_tensor(
                        po[:], sgv[:, :, 3], 64.0, pb[:],
                        op0=ALU.mult, op1=ALU.add,
                    )
                    for o in range(2):
                        nc.sync.dma_start(
                            outp.ap()[img, o, 30 * bb : 30 * bb + 30, :],
                            po[30 * o : 30 * o + 30, :],
                        )

    nc.compile()
    return nc


_CACHE = {}


def _get_program(n_img, A, B):
    key = (n_img, A, B)
    if key not in _CACHE:
        _CACHE[key] = _build_program(n_img, A, B)
    return _CACHE[key]


def _host_l1_pack(x, w1):
    """conv(x, sign(w1)) > 0, bit-packed as [n, 510, 16, 64] uint8.

    Cache-blocked: 30-row chunks keep the im2col slab + gemm output in
    L2, roughly halving the wall time vs one big im2col."""
    n = x.shape[0]
    W = np.sign(w1).astype(np.float32).reshape(16, 27)
    R = 30
    packed = np.empty((n, 510, 16, 64), np.uint8)
    col = np.empty((27, R * 510), np.float32)
    for i in range(n):
        xn = x[i]
        for r0 in range(0, 510, R):
            j = 0
            for c in range(3):
                for dy in range(3):
                    for dx in range(3):
                        col[j] = xn[c, r0 + dy : r0 + dy + R, dx : dx + 510].reshape(-1)
                        j += 1
            y = W @ col
            b = (y > 0).reshape(16, R, 510)
            packed[i, r0 : r0 + R] = np.packbits(b.transpose(1, 0, 2), axis=-1)
    return packed


# decode LUT: int8 byte p = s0 + 4*s1 + 16*s2 + 64*s3 (balanced quaternary)
_LUT = np.zeros((256, 4), np.float32)
for _s3 in (-1, 0, 1):
    for _s2 in (-1, 0, 1):
        for _s1 in (-1, 0, 1):
            for _s0 in (-1, 0, 1):
                _p = _s0 + 4 * _s1 + 16 * _s2 + 64 * _s3
                _LUT[_p + 128] = (_s0, _s1, _s2, _s3)

last_results = None


def kernel(inputs, w1, w2, w3):
    global last_results
    x = np.asarray(inputs, np.float32)
    n, _, A, B = x.shape
    per = n // N_CORES
    nc = _get_program(per, A, B)
    t2, t3 = _toeplitz_weights(
        np.asarray(w2, np.float32), np.asarray(w3, np.float32)
    )
    packed = _host_l1_pack(x, np.asarray(w1, np.float32))
    maps = [
        {"s1p": packed[per * i : per * (i + 1)], "t2w": t2, "t3w": t3}
        for i in range(N_CORES)
    ]
    res = run_bass_kernel_spmd(nc, maps, core_ids=list(range(N_CORES)))
    last_results = res
    a3, b3 = A - 6, B - 6
    out = np.empty((n, 2, a3, b3), np.float32)
    for i, r in enumerate(res.results):
        pk = r["outp"]  # [per, 2, 510, 127] int8
        idx = pk.astype(np.int16) + 128
        vals = _LUT[idx]  # [per, 2, 510, 127, 4]
        vals = vals.reshape(per, 2, pk.shape[2], -1)
        out[per * i : per * (i + 1)] = vals[:, :, :a3, :b3]
    return out.reshape(n, -1)
